# revision 8
# baseline (speedup 1.0000x reference)
"""Trainium2 Bass kernel: 12-head attention with relative position bias.

Reference computation (B=64, N=197, DIM=768, H=12, HD=64):
    qkv = x @ Wqkv.T + [q_bias, 0, v_bias]
    q, k, v = split(qkv); q *= HD**-0.5
    attn = softmax(q @ k.T + rel_table[rel_index].T)   # bias per head
    out = (attn @ v) reshaped -> @ Wproj.T + bproj

Strategy: pure data-parallel over batch (8 batches per NeuronCore x 8 cores,
no collectives). All matmuls in bf16 with fp32 PSUM accumulation. Attention
is computed transposed (attnT = k q^T, [keys, queries]) so attn @ v needs no
transpose; softmax uses no max-subtraction (|logits| < 3 for this operator's
input distribution) with per-query sums obtained by appending a ones column
to v; normalization is applied after AV via a DRAM-staged partition
broadcast of the reciprocal sums. The relative-position bias table gather is
performed on device with Toeplitz-structured negative-stride DMAs.
"""
import sys
sys.path.insert(0, '/opt/trn_rl_repo')
import itertools
import numpy as np
import ml_dtypes

import concourse.bass as bass
import concourse.mybir as mybir
from concourse import tile as _tile
from concourse.tile import TileContext, add_dep_helper
from concourse.vector_clock import ScopedClock
from concourse.bass_utils import run_bass_kernel_spmd

# ---------------------------------------------------------------------------
# Patches for this toolchain's one-sync-wait-per-instruction limit.
# The walrus build here rejects any instruction carrying more than one sem
# wait ("Too many sync wait commands"). Tile attaches multi-waits freely, so:
#  1. split the final drain's per-processor waits into single-wait nops;
#  2. after wait assignment, move every excess wait onto a fresh same-engine
#     NoOp inserted immediately before the instruction (engine program order
#     makes this equivalent; for DMAs it conservatively delays issue).
# ---------------------------------------------------------------------------
_counter = itertools.count()


def _drain_and_barrier_split(self, tick_clock, wait_clock):
    vc = tick_clock.global_clock
    for proc in range(len(vc)):
        t = vc[proc]
        if t > 0:
            sc = ScopedClock()
            sc.require_at_least(None, proc, t)
            nop_inst = self.nc.sync.nop(nofuse=True, hint="drain_split")
            wait_clock.add_sem_waits(nop_inst.ins, sc)
    self.nc.sync.drain()
    self.nc.all_engine_barrier()
    popped = self.nc._tile_sem_poison_stack.pop()
    assert popped is self._sem_poison
    self.nc.clear_and_free_semaphores(list(self.sems.allocated().values()))
    self.nc.all_engine_barrier()


_tile.TileContext._drain_and_barrier = _drain_and_barrier_split

_RealTileClockWait = _tile.TileClockWait
if getattr(_RealTileClockWait, "_is_split_wrapper", False):  # re-import safety
    _RealTileClockWait = _RealTileClockWait._real


def _split_excess_waits(ordered):
    for bb_name, insts in ordered.items():
        out = []
        changed = False
        for inst in insts:
            si = inst.sync_info
            waits = list(si.on_wait) if si is not None and si.on_wait else []
            if len(waits) > 1:
                changed = True
                for w in waits[:-1]:
                    nop = mybir.InstNoOp(
                        name=f"waitsplit_{next(_counter)}", engine=inst.engine)
                    nop.sync_info = mybir.SyncInfo(on_wait=[w], on_update=[])
                    nop.bass_nofuse = True
                    out.append(nop)
                inst.sync_info = mybir.SyncInfo(
                    on_wait=[waits[-1]],
                    on_update=list(si.on_update) if si.on_update else [])
            out.append(inst)
        if changed:
            insts[:] = out


class _TileClockWaitSplit:
    _is_split_wrapper = True
    _real = _RealTileClockWait

    def __init__(self, *args, **kwargs):
        self._inner = _RealTileClockWait(*args, **kwargs)
        self._ordered = args[1] if len(args) > 1 else kwargs["ordered_instructions_by_block"]

    def __getattr__(self, k):
        return getattr(self._inner, k)

    def assign_waits(self, bb_name):
        r = self._inner.assign_waits(bb_name)
        _split_excess_waits(self._ordered)
        return r


_tile.TileClockWait = _TileClockWaitSplit

# ---------------------------------------------------------------------------
# Problem constants (hardcoded; kernel.py must be self-contained)
# ---------------------------------------------------------------------------
B, N, DIM, H, HD = 64, 197, 768, 12, 64
NCORES = 8
BL = B // NCORES            # 8 batches per core
FPB = H * N                 # 2364: bias tile free pitch (12 head blocks of 197)
NQ0, NQ1 = 127, 70          # keys split: key 0 + Toeplitz rows rj 0..8 | rj 9..13
VW = HD + 1                 # 65: v block per head = 64 features + ones column
F32 = mybir.dt.float32
BF16 = mybir.dt.bfloat16
AF = mybir.ActivationFunctionType
ALU = mybir.AluOpType

_graph_cache = {}


import os
KSTAGE = int(os.environ.get('KSTAGE', '4'))
KATT = int(os.environ.get('KATT', '9'))


def _build_graph():
    nc = bass.Bass()
    xT = nc.declare_dram_parameter("xT", [BL, DIM, N], BF16, isOutput=False)
    wqkvT = nc.declare_dram_parameter("wqkvT", [DIM, 3 * DIM], BF16, isOutput=False)
    wprojT = nc.declare_dram_parameter("wprojT", [DIM, DIM], BF16, isOutput=False)
    qkbias = nc.declare_dram_parameter("qkbias", [128, 12], F32, isOutput=False)
    vbiasr = nc.declare_dram_parameter("vbiasr", [128, DIM], F32, isOutput=False)
    bprojc = nc.declare_dram_parameter("bprojc", [128, 6], F32, isOutput=False)
    tT = nc.declare_dram_parameter("tT", [H, 732], F32, isOutput=False)
    eB = nc.declare_dram_parameter("eB", [128, 3 * H], F32, isOutput=False)
    onesb = nc.declare_dram_parameter("onesb", [128, H], BF16, isOutput=False)
    yT = nc.declare_dram_parameter("yT", [BL, DIM, N], F32, isOutput=True)

    bstage = nc.dram_tensor("bias_stage", [N, FPB], F32)
    rstage = nc.dram_tensor("recip_stage", [BL, FPB], BF16)

    with nc.allow_low_precision(reason="bf16 compute validated: rel_err 3e-3 vs 2e-2 gate"), \
         TileContext(nc) as tc:
        with tc.tile_pool(name="const", bufs=1) as cpool, \
             tc.tile_pool(name="work", bufs=1) as wpool, \
             tc.tile_pool(name="small", bufs=3) as spool:

            # ---- resident constants -------------------------------------
            wq = [cpool.tile([128, 3 * DIM], BF16, tag=f"wq{c}", name=f"wq{c}") for c in range(6)]
            wp = [cpool.tile([128, DIM], BF16, tag=f"wp{c}", name=f"wp{c}") for c in range(6)]
            for c in range(6):
                nc.sync.dma_start(out=wq[c][:], in_=wqkvT[128 * c:128 * (c + 1), :])
                nc.sync.dma_start(out=wp[c][:], in_=wprojT[128 * c:128 * (c + 1), :])
            qkb = cpool.tile([128, 12], F32, tag="qkb")
            vbt = cpool.tile([128, DIM], F32, tag="vbt")
            bpc = cpool.tile([128, 6], F32, tag="bpc")
            ett = cpool.tile([128, 3 * H], F32, tag="ett")
            ons = cpool.tile([128, H], BF16, tag="ons")
            nc.sync.dma_start(out=qkb[:], in_=qkbias[:])
            nc.sync.dma_start(out=vbt[:], in_=vbiasr[:])
            nc.sync.dma_start(out=bpc[:], in_=bprojc[:])
            nc.sync.dma_start(out=ett[:], in_=eB[:])
            nc.sync.dma_start(out=ons[:], in_=onesb[:])

            # ---- relative-position bias build ---------------------------
            # bstage[key j, h*197 + query i]; host table grid is row-flipped
            # so value(rj,cj,ri,ci) = tG[13+rj-ri, 13+ci-cj] and the DMA dims
            # per (h, cj) strip are (rj:+27)(ri:-27)(ci:+1).
            staging = []
            for h in range(H):
                for cj in range(14):
                    dst = bass.AP(bstage, (1 + cj) * FPB + h * N + 1,
                                  [[14 * FPB, 14], [14, 14], [1, 14]])
                    src = bass.AP(tT, 732 * h + 364 - cj,
                                  [[27, 14], [-27, 14], [1, 14]])
                    staging.append(nc.sync.dma_start(out=dst, in_=src))
            b0t = cpool.tile([NQ0, FPB], F32, tag="b0t")
            b1t = cpool.tile([NQ1, FPB], F32, tag="b1t")
            ld0 = nc.sync.dma_start(out=b0t[:], in_=bstage[0:NQ0, :])
            ld1 = nc.sync.dma_start(out=b1t[:], in_=bstage[NQ0:N, :])
            for s in staging:
                add_dep_helper(ld0.ins, s.ins, sync=True, reason="bias staging")
                add_dep_helper(ld1.ins, s.ins, sync=True, reason="bias staging")
            for h in range(H):
                c0 = h * N
                # query-0 column (all keys) = t[729]
                nc.vector.tensor_copy(out=b0t[0:NQ0, c0:c0 + 1], in_=ett[0:NQ0, h * 3:h * 3 + 1])
                nc.vector.tensor_copy(out=b1t[0:NQ1, c0:c0 + 1], in_=ett[0:NQ1, h * 3:h * 3 + 1])
                # key-0 row: queries>=1 = t[730], query0 = t[731]
                nc.vector.tensor_copy(out=b0t[0:1, c0 + 1:c0 + N],
                                      in_=ett[0:1, h * 3 + 1:h * 3 + 2].to_broadcast([1, N - 1]))
                nc.vector.tensor_copy(out=b0t[0:1, c0:c0 + 1], in_=ett[0:1, h * 3 + 2:h * 3 + 3])

            # ---- load x (transposed, bf16): xall[c] = [128, BL*197] -----
            xall = [cpool.tile([128, BL * N], BF16, tag=f"x{c}", name=f"x{c}") for c in range(6)]
            for c in range(6):
                nc.sync.dma_start(
                    out=xall[c][:],
                    in_=bass.AP(xT, c * 128 * N,
                                [[N, 128], [DIM * N, BL], [1, N]]))

            # ---- qk projection: qkTm[m] = [128, BL*197] bf16 ------------
            # feature chunk m (0..5 q with 0.125 scale, 6..11 k), batches
            # pairwise packed along free (F=394) for one-bank psums.
            qkTm = [cpool.tile([128, BL * N], BF16, tag=f"qk{m}", name=f"qkTm{m}") for m in range(12)]
            with tc.tile_pool(name="ps_qk", bufs=8, space="PSUM") as pqk:
                for m in range(12):
                    pss = [pqk.tile([128, 2 * N], F32, tag="qkps", name=f"qkps{m}_{_}") for _ in range(4)]
                    for c in range(6):
                        for bp in range(4):
                            nc.tensor.matmul(
                                pss[bp][:],
                                wq[c][:, 128 * m:128 * (m + 1)],
                                xall[c][:, bp * 2 * N:(bp + 1) * 2 * N],
                                start=(c == 0), stop=(c == 5))
                    sc = 0.125 if m < 6 else 1.0
                    for bp in range(4):
                        nc.vector.tensor_scalar(
                            out=qkTm[m][:, bp * 2 * N:(bp + 1) * 2 * N],
                            in0=pss[bp][:], scalar1=sc,
                            scalar2=qkb[:, m:m + 1], op0=ALU.mult, op1=ALU.add)

            if KSTAGE < 2:
                return _early_out(nc, tc, spool, qkTm, yT)
            # ---- v projection: v_sb[b][nch] = [127|70, 780] bf16 --------
            # out = x_chunk.T @ Wv ([tokens, features]); per-head 65-wide
            # blocks (64 features + ones column) for the fused AV+sums.
            v_sb = [[cpool.tile([NQ0 if nch == 0 else NQ1, H * VW], BF16,
                                tag=f"v{b}_{nch}", name=f"v{b}_{nch}") for nch in range(2)] for b in range(BL)]
            with tc.tile_pool(name="ps_v", bufs=4, space="PSUM") as pv:
                for b in range(BL):
                    for nch in range(2):
                        nn_, nb = (NQ0, 0) if nch == 0 else (NQ1, NQ0)
                        ps = [pv.tile([NQ0, 384], F32, tag="vps", name=f"vps{b}_{nch}_{_}") for _ in range(2)]
                        for c in range(6):
                            for fh in range(2):
                                nc.tensor.matmul(
                                    ps[fh][0:nn_, :],
                                    xall[c][:, b * N + nb:b * N + nb + nn_],
                                    wq[c][:, 2 * DIM + 384 * fh:2 * DIM + 384 * (fh + 1)],
                                    start=(c == 0), stop=(c == 5))
                        for fh in range(2):
                            # +v_bias, cast bf16, into 65-wide head blocks
                            nc.vector.tensor_tensor(
                                out=bass.AP(v_sb[b][nch][:].tensor, VW * 6 * fh,
                                            [[H * VW, nn_], [VW, 6], [1, 64]]),
                                in0=ps[fh][0:nn_, :],
                                in1=vbt[0:nn_, 384 * fh:384 * (fh + 1)],
                                op=ALU.add)
                        # ones columns at 65h+64
                        nc.vector.tensor_copy(
                            out=bass.AP(v_sb[b][nch][:].tensor, 64, [[H * VW, nn_], [VW, H]]),
                            in_=ons[0:nn_, 0:H])

            if KSTAGE < 3:
                return _early_out(nc, tc, spool, qkTm, yT)
            # ---- attention + output projection --------------------------
            outT = [[cpool.tile([128, N], BF16, tag=f"o{b}_{cc}", name=f"outT{b}_{cc}") for cc in range(6)]
                    for b in range(BL)]
            with tc.tile_pool(name="ps_at", bufs=4, space="PSUM") as pat, \
                 tc.tile_pool(name="ps_av", bufs=3, space="PSUM") as pav, \
                 tc.tile_pool(name="dram_r", bufs=1, space="DRAM") as dr:
                for b in range(BL):
                    rrow = spool.tile([1, FPB], BF16, tag="rrow", bufs=2)
                    usb = [spool.tile([64, N], BF16, tag=f"u{h}", name=f"usb{h}", bufs=2) for h in range(H)]
                    for hp in range(6):
                        h0 = 2 * hp
                        mq, mk = hp, 6 + hp
                        for hh in range(2):
                            h = h0 + hh
                            rb = hh * 64
                            # per-head psum: key-chunks side by side (one bank;
                            # mixing row-groups in one bank crashes the DMA/PE)
                            psh = pat.tile([NQ0, 2 * N], F32, tag="atps", name=f"atps{b}_{h}")
                            nc.tensor.matmul(
                                psh[0:NQ0, 0:N],
                                qkTm[mk][rb:rb + 64, b * N:b * N + NQ0],
                                qkTm[mq][rb:rb + 64, b * N:b * N + N],
                                start=True, stop=True)
                            nc.tensor.matmul(
                                psh[0:NQ1, N:2 * N],
                                qkTm[mk][rb:rb + 64, b * N + NQ0:b * N + N],
                                qkTm[mq][rb:rb + 64, b * N:b * N + N],
                                start=True, stop=True)
                            # + rel-pos bias (in place in psum)
                            nc.vector.tensor_tensor(out=psh[0:NQ0, 0:N], in0=psh[0:NQ0, 0:N],
                                                    in1=b0t[0:NQ0, h * N:(h + 1) * N], op=ALU.add)
                            nc.vector.tensor_tensor(out=psh[0:NQ1, N:2 * N], in0=psh[0:NQ1, N:2 * N],
                                                    in1=b1t[0:NQ1, h * N:(h + 1) * N], op=ALU.add)
                            # exp (no max subtraction; |logits| < 3); the
                            # [70:127, N:2N] region is unused garbage
                            eh = spool.tile([NQ0, 2 * N], BF16, tag="eh", name=f"eh{b}_{h}")
                            nc.scalar.activation(out=eh[:], in_=psh[:], func=AF.Exp)
                            # AV with fused sums (ones column -> row 64)
                            pa = pav.tile([VW, N], F32, tag="avps", name=f"avps{b}_{h}")
                            nc.tensor.matmul(pa[:], v_sb[b][0][:, h * VW:(h + 1) * VW],
                                             eh[0:NQ0, 0:N], start=True, stop=False)
                            nc.tensor.matmul(pa[:], v_sb[b][1][:, h * VW:(h + 1) * VW],
                                             eh[0:NQ1, N:2 * N], start=False, stop=True)
                            nc.vector.reciprocal(out=rrow[0:1, h * N:(h + 1) * N],
                                                 in_=pa[64:65, :])
                            nc.vector.tensor_copy(out=usb[h][:], in_=pa[0:64, :])
                    if KATT < 5:
                        return _early_out(nc, tc, spool, qkTm, yT, dump=usb[0][0:64, 0:N])
                    # normalize: reciprocal row -> DRAM -> partition broadcast
                    rwr = nc.sync.dma_start(out=rstage[b:b + 1, :], in_=rrow[0:1, :])
                    rbc = spool.tile([64, FPB], BF16, tag="rbc", bufs=2)
                    rrd = nc.sync.dma_start(out=rbc[:], in_=rstage[b:b + 1, :].to_broadcast((64, FPB)))
                    add_dep_helper(rrd.ins, rwr.ins, sync=True, reason="recip staging")
                    for cc in range(6):
                        h0 = 2 * cc
                        nc.vector.tensor_tensor(out=outT[b][cc][0:64, :], in0=usb[h0][:],
                                                in1=rbc[:, h0 * N:(h0 + 1) * N], op=ALU.mult)
                        tmp = spool.tile([64, N], BF16, tag="tmp")
                        nc.vector.tensor_tensor(out=tmp[:], in0=usb[h0 + 1][:],
                                                in1=rbc[:, (h0 + 1) * N:(h0 + 2) * N], op=ALU.mult)
                        # partition shift 0..63 -> 64..127 (engines cannot)
                        nc.sync.dma_start(out=outT[b][cc][64:128, :], in_=tmp[:])

            if KATT < 6:
                return _early_out(nc, tc, spool, qkTm, yT, dump=outT[0][0][0:64, 0:N])
            if KSTAGE < 4:
                return _early_out(nc, tc, spool, qkTm, yT)
            # ---- output projection: yT[b] = Wproj @ outT ----------------
            with tc.tile_pool(name="ps_pj", bufs=8, space="PSUM") as ppj:
                for mp in range(6):
                    pss = [ppj.tile([128, N], F32, tag="pjps", name=f"pjps{mp}_{_}") for _ in range(BL)]
                    for c in range(6):
                        for b in range(BL):
                            nc.tensor.matmul(pss[b][:], wp[c][:, 128 * mp:128 * (mp + 1)],
                                             outT[b][c][:], start=(c == 0), stop=(c == 5))
                    for b in range(BL):
                        ysb = spool.tile([128, N], F32, tag="ysb")
                        nc.vector.tensor_scalar(out=ysb[:], in0=pss[b][:],
                                                scalar1=bpc[:, mp:mp + 1], scalar2=None,
                                                op0=ALU.add)
                        nc.sync.dma_start(
                            out=bass.AP(yT, b * DIM * N + mp * 128 * N, [[N, 128], [1, N]]),
                            in_=ysb[:])
    return nc


def _early_out(nc, tc, spool, qkTm, yT, dump=None):
    ysb = spool.tile([128, N], F32, tag="ysb", name="ysb_early")
    src = dump if dump is not None else qkTm[0][:, 0:N]
    nc.vector.tensor_copy(out=ysb[0:src.shape[0], 0:src.shape[1]], in_=src)
    nc.sync.dma_start(out=bass.AP(yT, 0, [[N, 128], [1, N]]), in_=ysb[:])
    return nc


def _prep_inputs(x, Wqkv, q_bias, v_bias, rel_table, Wproj, bproj, rel_index):
    bf = ml_dtypes.bfloat16
    xT = np.ascontiguousarray(np.asarray(x).transpose(0, 2, 1)).astype(bf)
    wqkvT = np.ascontiguousarray(np.asarray(Wqkv).T).astype(bf)
    wprojT = np.ascontiguousarray(np.asarray(Wproj).T).astype(bf)
    qb = np.asarray(q_bias) * (HD ** -0.5) * 0.0  # placeholder, replaced below
    # qk bias per 128-chunk: q chunks pre-scaled by HD**-0.5, k chunks zero
    qs = np.concatenate([np.asarray(q_bias) * (HD ** -0.5), np.zeros(DIM, np.float32)])
    qkbias = np.ascontiguousarray(qs.reshape(12, 128).T).astype(np.float32)
    vbiasr = np.tile(np.asarray(v_bias).reshape(1, DIM), (128, 1)).astype(np.float32)
    bprojc = np.ascontiguousarray(np.asarray(bproj).reshape(6, 128).T).astype(np.float32)
    tTh = np.ascontiguousarray(np.asarray(rel_table).T).astype(np.float32)
    tG = tTh.copy()
    tG[:, :729] = tTh[:, :729].reshape(H, 27, 27)[:, ::-1, :].reshape(H, 729)
    eB = np.tile(tTh[:, 729:732].reshape(1, 3 * H), (128, 1)).astype(np.float32)
    onesb = np.ones((128, H), dtype=bf)
    return xT, wqkvT, wprojT, qkbias, vbiasr, bprojc, tG, eB, onesb


def run_sharded(inputs, trace=False):
    nc = _graph_cache.get("nc")
    if nc is None:
        nc = _build_graph()
        _graph_cache["nc"] = nc
    xT, wqkvT, wprojT, qkbias, vbiasr, bprojc, tG, eB, onesb = _prep_inputs(**inputs)
    in_maps = []
    for i in range(NCORES):
        in_maps.append({
            "xT": np.ascontiguousarray(xT[i * BL:(i + 1) * BL]),
            "wqkvT": wqkvT, "wprojT": wprojT, "qkbias": qkbias,
            "vbiasr": vbiasr, "bprojc": bprojc, "tT": tG, "eB": eB,
            "onesb": onesb,
        })
    res = run_bass_kernel_spmd(nc, in_maps, list(range(NCORES)), trace=trace)
    outs = []
    for i in range(NCORES):
        ytc = np.asarray(res.results[i]["yT"])          # [BL, DIM, N] f32
        outs.append(ytc.transpose(0, 2, 1))             # [BL, N, DIM]
    y = np.concatenate(outs, axis=0).astype(np.float32)
    return y, res


def kernel(**inputs) -> np.ndarray:
    y, _ = run_sharded(inputs, trace=False)
    return y
